# revision 16
# baseline (speedup 1.0000x reference)
"""Causal self-attention (GPT-2 block) for Trainium2, 8 NeuronCores.

Sharding: core = 2*batch + head_group. Each of the 8 cores handles one of
B=4 batches and one group of 8 of the 16 heads (Megatron column-split of
the QKV weights, row-split of the proj weights). The two head-group
partial proj outputs per batch are summed on the host; the V-bias and
proj-bias terms are folded into a single host-side additive correction
(softmax rows sum to 1, so attn @ (1 x bv) == bv broadcast).

On-core layout (everything 4-byte, matmul operands are float32r so the
PE runs at full 1-cycle/row speed):
  xT    [128, 8, S]   x transposed via PE transpose-mode (per s-chunk)
  QT/KT [128, 4, S]   feature-major: partition p, slice j <-> feature j*128+p
                      head h lives at partitions (h%2)*64.. , slice h//2
  V     [128, 16, 8, 65]  natural [s, feat] per head + ones column (row sums)
  attnT [128, 4, S]   attention output, feature-major (proj stationary)

Attention per (head, 512-wide q-chunk): scoresT blocks [128 k, <=512 q]
via KT-block.T @ QT (contract 64, two heads packed in the PE array via
partition halves 0/64), additive -1e30 causal mask on the 128-col
diagonal corner of the PSUM scores, exp on ScalarE with the 1/8 scale
folded in, PV + row-sums via the V ones-column, normalize with DVE
reciprocal + gpsimd partition broadcast, then SBUF->SBUF DMA into
attnT's partition half.
"""

import numpy as np

import concourse.bass as bass
import concourse.tile as tile
from concourse import bacc, mybir
from concourse.bass_utils import run_bass_kernel_spmd
from concourse.masks import make_identity, make_lower_triangular

# Problem shape (fixed by the harness contract).
B, S, D, H, HD = 4, 2048, 1024, 16, 64
NCORES = 8
HG = 8                # heads per core
FG = HG * HD          # 512 features per head group
P = 128
DB = D // P           # 8 contraction blocks
FBN = FG // P         # 4 feature blocks
SC = 512              # sequence chunk
NQ = S // SC          # 4
NKB = S // P          # 16 key blocks
F32 = mybir.dt.float32
F32R = mybir.dt.float32r
BF16 = mybir.dt.bfloat16
import os
DT_MM = BF16 if os.environ.get("KERNEL_DT", "f32r") == "bf16" else F32R
EXP = mybir.ActivationFunctionType.Exp
SCALE = 1.0 / float(HD) ** 0.5
MASKVAL = -1e30


def _attention_pair(nc, hA, hB, q, QT, KT, V, attnT, addmask,
                    sxp, nrm, psst, psout, drp, dbg2=None):
    nrmraw, nrmbc = nrm
    """Emit one q-chunk of attention for a pair of heads (partition halves
    0/64 so their contract-64 matmuls can co-occupy the PE array)."""
    # Block list: diagonal jj=0 first (start=True covers the full 512 cols),
    # then the full causal band, then the narrowing diagonal blocks.
    blocks = [(4 * q, 0)] + [(kb, None) for kb in range(4 * q)] + \
             [(4 * q + jj, jj) for jj in (1, 2, 3)]
    heads = []
    for h in (hA, hB):
        out_ps = psout.tile([65, SC], F32, tag="outps")
        heads.append((h, (h % 2) * 64, h // 2, out_ps))

    nblk = len(blocks)
    for i, (kb, jj) in enumerate(blocks):
        off = 0 if jj is None else jj * P
        w = SC - off
        sts = []
        for h, pb, j, out_ps in heads:
            st = psst.tile([P, SC], F32, tag="stps")
            nc.tensor.matmul(
                st[:, :w],
                KT[pb:pb + 64, j, kb * P:(kb + 1) * P],
                QT[pb:pb + 64, j, q * SC + off:(q + 1) * SC],
                start=True, stop=True, tile_position=(pb, 0))
            if jj is not None:
                # Causal mask: only the leading 128 cols of a diagonal
                # block straddle the diagonal.
                nc.vector.tensor_add(st[:, :P], st[:, :P], addmask)
            if dbg2 is not None and h == hA and i == 0:
                stc = nrmraw.tile([P, SC], F32, tag="dbg_stc")
                nc.vector.tensor_copy(stc, st)
                nc.sync.dma_start(dbg2["st00"].ap(), stc)
            sts.append(st)
        sxs = []
        for (h, pb, j, out_ps), st in zip(heads, sts):
            sx = sxp.tile([P, SC], DT_MM, tag="sx")
            nc.scalar.activation(sx[:, :w], st[:, :w], EXP, scale=SCALE)
            if dbg2 is not None and h == hA and i == 0:
                nc.sync.dma_start(dbg2["sx00"].ap(), sx)
            sxs.append(sx)
        for (h, pb, j, out_ps), sx in zip(heads, sxs):
            nc.tensor.matmul(
                out_ps[:, off:], V[:, kb, h, :], sx[:, :w],
                start=(i == 0), stop=(i == nblk - 1))

    raws = []
    for h, pb, j, out_ps in heads:
        raw = nrmraw.tile([65, SC], F32, tag="raw")
        nc.vector.tensor_copy(raw, out_ps)
        if dbg2 is not None and h == hA:
            nc.sync.dma_start(dbg2["raw00"].ap(), raw)
        raws.append(raw)
    for (h, pb, j, out_ps), raw in zip(heads, raws):
        # Reciprocal on one partition is ~3.3us (single lane) and blocks
        # the DVE FIFO; scatter the 512 sums across 128 partitions first.
        rsh = nrmbc.tile([P, SC // P], F32, tag="rsh")
        nc.sync.dma_start(rsh, raw[64:65, :])
        nc.vector.reciprocal(rsh, rsh)
        rdram = drp.tile([1, SC], F32, tag="rdram")
        nc.sync.dma_start(rdram, rsh)
        rb = nrmbc.tile([64, SC], F32, tag="rb")
        nc.sync.dma_start(rb, rdram.to_broadcast([64, SC]))
        if dbg2 is not None and h == hA:
            nc.sync.dma_start(dbg2["rb00"].ap(), rb)
        stg = nrmbc.tile([64, SC], DT_MM, tag="stg")
        nc.vector.tensor_mul(stg, raw[0:64, :], rb)
        nc.sync.dma_start(attnT[pb:pb + 64, j, q * SC:(q + 1) * SC], stg)


def _body(tc, x_d, wq_d, wk_d, wv_d, wp_d, bq_d, bk_d, out_d, dbg=None,
          dbg2=None):
    nc = tc.nc
    with tc.tile_pool(name="persist", bufs=1) as persist:
        ident = persist.tile([P, P], F32)
        make_identity(nc, ident)
        addmask = persist.tile([P, P], F32)
        make_lower_triangular(nc, addmask, val=MASKVAL, diag=False)
        bq_sb = persist.tile([P, FBN], F32)
        bk_sb = persist.tile([P, FBN], F32)
        nc.sync.dma_start(bq_sb, bq_d.ap().rearrange("(j p) -> p j", p=P))
        nc.sync.dma_start(bk_sb, bk_d.ap().rearrange("(j p) -> p j", p=P))

        QT = persist.tile([P, FBN, S], DT_MM)
        KT = persist.tile([P, FBN, S], DT_MM)
        V = persist.tile([P, NKB, HG, HD + 1], DT_MM)
        ones_col = persist.tile([P, 1], F32)
        nc.vector.memset(ones_col, 1.0)
        nc.vector.tensor_copy(V[:, :, :, HD],
                              ones_col.to_broadcast([P, NKB, HG]))

        # ---- Phase 1: transpose x, QKV projections ----
        with (
            tc.tile_pool(name="ph1", bufs=1) as ph1,
            tc.tile_pool(name="xin", bufs=3) as xinp,
            tc.tile_pool(name="xtp", bufs=2) as xtp,
            tc.tile_pool(name="ps1", bufs=2, space="PSUM") as ps1,
            tc.tile_pool(name="pst", bufs=2, space="PSUM") as pst,
        ):
            wq_sb = ph1.tile([P, DB, FG], DT_MM)
            wk_sb = ph1.tile([P, DB, FG], DT_MM)
            wv_sb = ph1.tile([P, DB, FG], DT_MM)
            for w_sb, w_d in ((wq_sb, wq_d), (wk_sb, wk_d), (wv_sb, wv_d)):
                nc.sync.dma_start(
                    w_sb, w_d.ap().rearrange("(db p) f -> p db f", p=P))

            def transpose_chunk(c, xt):
                # Returns a list of emit-thunks (one per PE transpose) so the
                # caller can interleave them between QKV matmuls; a burst of
                # transpose-mode ops reads as idle to the PE HAM and drops
                # the clock to 1.2 GHz.
                thunks = []
                for sb in range(SC // P):
                    xin = xinp.tile([P, D], F32, tag="xin")
                    s0 = c * SC + sb * P
                    nc.sync.dma_start(xin, x_d.ap()[s0:s0 + P, :])
                    for db in range(DB):
                        def t(sb=sb, db=db, xin=xin):
                            pt = pst.tile([P, P], F32, tag="pt")
                            nc.tensor.transpose(
                                pt, xin[:, db * P:(db + 1) * P], ident)
                            nc.any.tensor_copy(
                                xt[:, db, sb * P:(sb + 1) * P], pt)
                        thunks.append(t)
                return thunks

            xts = [xtp.tile([P, DB, SC], DT_MM, tag="xt", name=f"xt{c}")
                   for c in range(NQ)]
            for t in transpose_chunk(0, xts[0]):
                t()
            for c in range(NQ):
                xt = xts[c]
                tr_next = (transpose_chunk(c + 1, xts[c + 1])
                           if c + 1 < NQ else [])
                tri = iter(tr_next)

                def drip(n):
                    for _ in range(n):
                        t = next(tri, None)
                        if t is not None:
                            t()

                # Q and K -> transposed feature-major layout, bias added.
                for w_sb, T, b_sb in ((wq_sb, QT, bq_sb), (wk_sb, KT, bk_sb)):
                    for fb in range(FBN):
                        ps = ps1.tile([P, SC], F32, tag="qkps")
                        for db in range(DB):
                            nc.tensor.matmul(
                                ps,
                                w_sb[:, db, fb * P:(fb + 1) * P],
                                xt[:, db, :],
                                start=(db == 0), stop=(db == DB - 1))
                            drip(1 if db % 2 else 0)
                        nc.vector.tensor_scalar_add(
                            T[:, fb, c * SC:(c + 1) * SC], ps,
                            b_sb[:, fb:fb + 1])
                # V -> natural [s, feat] layout (no bias: folded on host).
                for sb in range(SC // P):
                    kb = c * (SC // P) + sb
                    ps = ps1.tile([P, SC], F32, tag="qkps")
                    for db in range(DB):
                        nc.tensor.matmul(
                            ps,
                            xt[:, db, sb * P:(sb + 1) * P],
                            wv_sb[:, db, :],
                            start=(db == 0), stop=(db == DB - 1))
                    nc.vector.tensor_copy(
                        V[:, kb, :, 0:HD],
                        ps.rearrange("p (h c) -> p h c", h=HG))
                drip(32)

        # ---- Phase 2+3 interleaved: attention per q-chunk, then proj ----
        with (
            tc.tile_pool(name="ph23", bufs=1) as ph23,
            tc.tile_pool(name="sxp", bufs=4) as sxp,
            tc.tile_pool(name="nrmraw", bufs=8) as nrmraw,
            tc.tile_pool(name="nrmbc", bufs=5) as nrmbc,
            tc.tile_pool(name="ogp", bufs=2) as ogp,
            tc.tile_pool(name="psst", bufs=3, space="PSUM") as psst,
            tc.tile_pool(name="psout", bufs=3, space="PSUM") as psout,
            tc.tile_pool(name="pspr", bufs=2, space="PSUM") as pspr,
            tc.tile_pool(name="drp", bufs=8, space="DRAM") as drp,
        ):
            attnT = ph23.tile([P, FBN, S], DT_MM)
            wp_sb = ph23.tile([P, FBN, D], DT_MM)
            nc.sync.dma_start(
                wp_sb, wp_d.ap().rearrange("(j p) n -> p j n", p=P))

            def proj_chunk(q, only_sb=None):
                for sb in range(SC // P):
                    if only_sb is not None and sb != only_sb:
                        continue
                    sblk = q * (SC // P) + sb
                    og = ogp.tile([P, D], F32, tag="og")
                    for half in range(2):
                        ps = pspr.tile([P, D // 2], F32, tag="prps")
                        n0 = half * (D // 2)
                        for j in range(FBN):
                            nc.tensor.matmul(
                                ps,
                                attnT[:, j, sblk * P:(sblk + 1) * P],
                                wp_sb[:, j, n0:n0 + D // 2],
                                start=(j == 0), stop=(j == FBN - 1))
                        nc.any.tensor_copy(og[:, n0:n0 + D // 2], ps)
                    nc.sync.dma_start(out_d.ap()[sblk * P:(sblk + 1) * P, :],
                                      og)

            for q in range(NQ):
                for hp in range(HG // 2):
                    _attention_pair(nc, 2 * hp, 2 * hp + 1, q, QT, KT, V,
                                    attnT, addmask, sxp, (nrmraw, nrmbc),
                                    psst, psout, drp,
                                    dbg2=(dbg2 if (hp == 0 and q == 0)
                                          else None))
                    # Weave the previous chunk's proj between attention
                    # pairs: its contract-128 matmuls keep the PE HAM warm.
                    if q > 0:
                        proj_chunk(q - 1, only_sb=hp)
            proj_chunk(NQ - 1)

            if dbg is not None:
                qt_d, kt_d, v_d, at_d = dbg
                nc.sync.dma_start(qt_d.ap(), QT)
                nc.sync.dma_start(kt_d.ap(), KT)
                nc.sync.dma_start(v_d.ap(), V)
                nc.sync.dma_start(at_d.ap(), attnT)


def build_nc():
    nc = bacc.Bacc("TRN2", target_bir_lowering=False)
    x_d = nc.dram_tensor("x", [S, D], F32, kind="ExternalInput")
    wq_d = nc.dram_tensor("wq", [D, FG], DT_MM, kind="ExternalInput")
    wk_d = nc.dram_tensor("wk", [D, FG], DT_MM, kind="ExternalInput")
    wv_d = nc.dram_tensor("wv", [D, FG], DT_MM, kind="ExternalInput")
    wp_d = nc.dram_tensor("wp", [FG, D], DT_MM, kind="ExternalInput")
    bq_d = nc.dram_tensor("bq", [FG], F32, kind="ExternalInput")
    bk_d = nc.dram_tensor("bk", [FG], F32, kind="ExternalInput")
    out_d = nc.dram_tensor("out", [S, D], F32, kind="ExternalOutput")
    with tile.TileContext(nc) as tc:
        _body(tc, x_d, wq_d, wk_d, wv_d, wp_d, bq_d, bk_d, out_d)
    nc.compile()
    return nc


_NC = None


def _get_nc():
    global _NC
    if _NC is None:
        _NC = build_nc()
    return _NC


def make_in_maps(hs, w, bvec, pw):
    import ml_dtypes
    wdt = (ml_dtypes.bfloat16 if DT_MM == BF16 else np.float32)
    in_maps = []
    for core in range(NCORES):
        b, g = divmod(core, 2)
        lo, hi = g * FG, (g + 1) * FG
        in_maps.append({
            "x": np.ascontiguousarray(hs[b]),
            "wq": np.ascontiguousarray(w[:, lo:hi]).astype(wdt),
            "wk": np.ascontiguousarray(w[:, D + lo:D + hi]).astype(wdt),
            "wv": np.ascontiguousarray(w[:, 2 * D + lo:2 * D + hi]).astype(wdt),
            "wp": np.ascontiguousarray(pw[lo:hi, :]).astype(wdt),
            "bq": np.ascontiguousarray(bvec[lo:hi]),
            "bk": np.ascontiguousarray(bvec[D + lo:D + hi]),
        })
    return in_maps


def combine(parts, bvec, pw, pb):
    bv = bvec[2 * D:3 * D].astype(np.float64)
    corr = (bv @ pw.astype(np.float64) + pb.astype(np.float64)).astype(
        np.float32)
    out = np.empty((B, S, D), np.float32)
    for b in range(B):
        out[b] = parts[2 * b] + parts[2 * b + 1] + corr
    return out


def kernel(hidden_states, c_attn_w, c_attn_b, c_proj_w, c_proj_b,
           **run_kwargs):
    hs = np.asarray(hidden_states, dtype=np.float32)
    w = np.asarray(c_attn_w, dtype=np.float32)
    bvec = np.asarray(c_attn_b, dtype=np.float32)
    pw = np.asarray(c_proj_w, dtype=np.float32)
    pb = np.asarray(c_proj_b, dtype=np.float32)
    nc = _get_nc()
    res = run_bass_kernel_spmd(nc, make_in_maps(hs, w, bvec, pw),
                               core_ids=list(range(NCORES)), **run_kwargs)
    parts = [res.results[i]["out"] for i in range(NCORES)]
    out = combine(parts, bvec, pw, pb)
    if run_kwargs:
        return out, res
    return out
